# revision 20
# baseline (speedup 1.0000x reference)
"""Trainium2 Bass kernel for nn_CausalLinearAttention (B=4, T=4096, D=1024, H=16).

Sharding: 8 cores = 4 batches x 2 head-shards (8 heads each). Per core, four
dense phases keep the PE streaming back-to-back N=512 matmuls (HAM warm):
  A: K-local projection + feature for all T, per-token local head-sum s_own,
     ONE AllReduce (pairwise) for the cross-shard head-cumsum base.
  B: Q projection + feature, V projection (token-major) for all T.
  C: normalizer fold: head-prefix sums R built in 4 PSUM banks by pure-MM
     accumulation (no PE<->DVE ping-pong), z = 1/(q . R) folded into Q.
  D: chunked causal scan; K transposed token-major via DMA-XBAR transposes
     (off the PE), per-head 64x64 state accumulated in a dedicated PSUM bank,
     output projection interleaved per 128-token chunk as dense filler.
Host sums the two head-shard partials per batch and adds the bias.

Self-contained: hardcodes shapes; only needs the concourse repo on sys.path.
"""
import sys

for _p in ("/opt/trn_rl_repo", "/root/.axon_site/_ro/trn_rl_repo"):
    if _p not in sys.path:
        sys.path.append(_p)

import numpy as np
import ml_dtypes

import concourse.bass as bass
from concourse import bacc, mybir
from concourse.bass_utils import run_bass_kernel_spmd
from concourse.tile import TileContext

bf16 = mybir.dt.bfloat16
f32 = mybir.dt.float32
AF = mybir.ActivationFunctionType
ALU = mybir.AluOpType

B, T, D = 4, 4096, 1024
H, DH = 16, 64
KD = D // 128      # 8 contraction tiles over d_model
NT = T // 128      # 32 token tiles
NB = 4             # 1024-token blocks
TB = T // NB
NCH = T // 512     # 8 512-token chunks


def build_nc(debug=False, phases="abcd", use_xbar=True, dvar="full"):
    nc = bacc.Bacc("TRN2", target_bir_lowering=False, debug=debug)

    xt = nc.dram_tensor("xt", [128, KD, T], bf16, kind="ExternalInput")
    wq = nc.dram_tensor("wq", [128, KD, 512], bf16, kind="ExternalInput")
    wk = nc.dram_tensor("wk", [128, KD, 512], bf16, kind="ExternalInput")
    wv = nc.dram_tensor("wv", [128, KD, 512], bf16, kind="ExternalInput")
    wo = nc.dram_tensor("wo", [128, 4, D], bf16, kind="ExternalInput")
    maskt_d = nc.dram_tensor("maskt", [128, 128], bf16, kind="ExternalInput")
    maskd_d = nc.dram_tensor("maskd", [128, 256], f32, kind="ExternalInput")
    sel_all_d = nc.dram_tensor("sel_all", [128, 128], bf16, kind="ExternalInput")
    sel_last_d = nc.dram_tensor("sel_last", [128, 128], bf16, kind="ExternalInput")
    sel_f_d = nc.dram_tensor("sel_f", [128, 128], bf16, kind="ExternalInput")
    sel_sum_d = nc.dram_tensor("sel_sum", [128, 128], bf16, kind="ExternalInput")
    zsel_d = nc.dram_tensor("zsel", [128, 128], bf16, kind="ExternalInput")
    ident_d = nc.dram_tensor("ident", [128, 128], bf16, kind="ExternalInput")
    y = nc.dram_tensor("y", [T, D], bf16, kind="ExternalOutput")

    with TileContext(nc) as tc:
      with tc.tile_pool(name="consts", bufs=1) as consts, \
           tc.tile_pool(name="big", bufs=1) as big, \
           tc.tile_pool(name="dram", bufs=1, space="DRAM") as dram:
        wk_s = consts.tile([128, KD, 512], bf16, tag="wk_s")
        wq_s = consts.tile([128, KD, 512], bf16, tag="wq_s")
        wv_s = consts.tile([128, KD, 512], bf16, tag="wv_s")
        wo_s = consts.tile([128, 4, D], bf16, tag="wo_s")
        mask_s = consts.tile([128, 128], bf16, tag="mask_s")
        maskd_s = consts.tile([128, 256], f32, tag="maskd_s")
        sel_all_s = consts.tile([128, 128], bf16, tag="sel_all_s")
        sel_last_s = consts.tile([128, 128], bf16, tag="sel_last_s")
        sel_f_s = consts.tile([128, 128], bf16, tag="sel_f_s")
        sel_sum_s = consts.tile([128, 128], bf16, tag="sel_sum_s")
        zsel_s = consts.tile([128, 128], bf16, tag="zsel_s")
        ident_s = consts.tile([128, 128], bf16, tag="ident_s")
        for dst, src in ((wk_s, wk), (wq_s, wq), (wv_s, wv), (wo_s, wo),
                         (mask_s, maskt_d), (maskd_s, maskd_d), (sel_all_s, sel_all_d),
                         (sel_last_s, sel_last_d), (sel_f_s, sel_f_d),
                         (sel_sum_s, sel_sum_d), (zsel_s, zsel_d),
                         (ident_s, ident_d)):
            nc.gpsimd.dma_start(out=dst, in_=src[tuple([slice(None)] * len(src.shape))])

        negone = consts.tile([128, 1], f32, tag="negone")
        nc.vector.memset(negone, -1.0)
        kt = big.tile([128, 4, T], bf16, tag="kt")
        qtb = big.tile([128, 4, T], bf16, tag="qtb")
        vnat = big.tile([128, NT, 512], bf16, tag="vnat")
        s_own = big.tile([64, T], bf16, tag="s_own")
        s_tot = big.tile([64, T], bf16, tag="s_tot")
        s_oth = big.tile([64, T], bf16, tag="s_oth")

        ktd = dram.tile([512, T], bf16, tag="ktd")
        cc_in = dram.tile([64, T], bf16, tag="cc_in")
        cc_out = dram.tile([64, T], bf16, tag="cc_out")

        # ================= Phases A & B =================
        with tc.tile_pool(name="xtp", bufs=2) as xtp, \
             tc.tile_pool(name="fe", bufs=3) as fe, \
             tc.tile_pool(name="psA", bufs=4, space="PSUM") as psA, \
             tc.tile_pool(name="psSO", bufs=2, space="PSUM") as psSO:

            def feature(psum_ap, out_ap):
                e = fe.tile([128, 512], f32, tag="e")
                nc.scalar.activation(e, psum_ap, AF.Exp)
                e2 = fe.tile([128, 512], bf16, tag="e2")
                nc.vector.tensor_scalar_min(e2, e, 1.0)
                nc.vector.scalar_tensor_tensor(out=out_ap, in0=psum_ap, scalar=0.0,
                                               in1=e2, op0=ALU.max, op1=ALU.add)

            def load_x(blk):
                xtb = xtp.tile([128, KD, TB], bf16, tag="xtb", name=f"xtb{blk}")
                for nch in range(2):
                    nc.sync.dma_start(out=xtb[:, :, nch * 512:(nch + 1) * 512],
                                      in_=xt[:, :, blk * TB + nch * 512: blk * TB + (nch + 1) * 512])
                return xtb

            def proj_block(blk, w_s, out_t, xtb):
                """feature(x @ W) for one 1024-token block, feat-major out."""
                for m in range(4):
                    pks = [psA.tile([128, 512], f32, tag="pk", name=f"pk{blk}_{m}_{i}")
                           for i in range(2)]
                    for k in range(KD):
                        for nch in range(2):
                            nc.tensor.matmul(pks[nch], w_s[:, k, m * 128:(m + 1) * 128],
                                             xtb[:, k, nch * 512:(nch + 1) * 512],
                                             start=(k == 0), stop=(k == KD - 1))
                    for nch in range(2):
                        csl = slice(blk * TB + nch * 512, blk * TB + (nch + 1) * 512)
                        feature(pks[nch], out_t[:, m, csl])
                return xtb

            # ---- Phase A: K + s_own, fire AllReduce
            xtb_cur = load_x(0)
            for blk in range(NB):
                xtb_nxt = load_x(blk + 1) if blk + 1 < NB else None
                proj_block(blk, wk_s, kt, xtb_cur)
                xtb_cur = xtb_nxt
                for nch in range(2):
                    csl = slice(blk * TB + nch * 512, blk * TB + (nch + 1) * 512)
                    so = psSO.tile([64, 512], f32, tag="so")
                    for m in range(4):
                        nc.tensor.matmul(so, sel_sum_s[:, 0:64], kt[:, m, csl],
                                         start=(m == 0), stop=(m == 3))
                    nc.scalar.copy(s_own[:, csl], so)
                bsl = slice(blk * TB, (blk + 1) * TB)
                nc.sync.dma_start(
                    out=ktd[:, bsl].rearrange("(m p) t -> p m t", p=128),
                    in_=kt[:, :, bsl])
            xtb_cur = load_x(0)
            nc.sync.dma_start(out=cc_in[:, :], in_=s_own)
            nc.gpsimd.collective_compute(
                "AllReduce", ALU.add,
                replica_groups=[[0, 1], [2, 3], [4, 5], [6, 7]],
                ins=[cc_in[:, :]], outs=[cc_out[:, :]])

            # ---- Phase B: Q + V
            for blk in range(NB):
                xtb_nxt = load_x(blk + 1) if blk + 1 < NB else None
                xtb = xtb_cur
                proj_block(blk, wq_s, qtb, xtb)
                xtb_cur = xtb_nxt
                for tt in range(8):
                    pv = psA.tile([128, 512], f32, tag="pk", name=f"pv{blk}_{tt}")
                    for k in range(KD):
                        nc.tensor.matmul(pv, xtb[:, k, tt * 128:(tt + 1) * 128],
                                         wv_s[:, k, :], start=(k == 0), stop=(k == KD - 1))
                    nc.scalar.copy(vnat[:, blk * 8 + tt, :], pv)

            # cc readback (sync queue: after all B loads so nothing stalls behind it)
            nc.sync.dma_start(out=s_tot, in_=cc_out[:, :])
            nc.vector.tensor_sub(s_oth, s_tot, s_own)

        if "c" not in phases:
            with tc.tile_pool(name="dummy", bufs=1) as dummy:
                yd = dummy.tile([128, D], bf16, tag="yd")
                nc.vector.memset(yd, 0.1)
                for tt in range(NT):
                    nc.sync.dma_start(out=y[tt * 128:(tt + 1) * 128, :], in_=yd)
            skip_cd = True
        else:
            skip_cd = False

        # ================= Phases C & D =================
        if not skip_cd:
          with tc.tile_pool(name="knatp", bufs=2) as knatp, \
             tc.tile_pool(name="tmpp", bufs=2) as tmpp, \
             tc.tile_pool(name="ztp", bufs=2) as ztp, \
             tc.tile_pool(name="attnp", bufs=2) as attnp, \
             tc.tile_pool(name="outp", bufs=2) as outp, \
             tc.tile_pool(name="sbfp", bufs=2) as sbfp, \
             tc.tile_pool(name="ystp", bufs=2) as ystp:

            knat_tiles = [None] * NB

            def emit_knat(blk):
                kn = knatp.tile([128, 8, 512], bf16, tag="knat", name=f"knat{blk}")
                for t8 in range(8):
                    tsl = slice(blk * TB + t8 * 128, blk * TB + (t8 + 1) * 128)
                    nc.sync.dma_start_transpose(out=kn[:, t8, :], in_=ktd[:, tsl])
                knat_tiles[blk] = kn

            if use_xbar:
                emit_knat(0)

            # ---- Phase C: z fold into Q
            with tc.tile_pool(name="psP", bufs=4, space="PSUM") as psP, \
                 tc.tile_pool(name="psZ", bufs=2, space="PSUM") as psZ:
                # software-pipelined by one chunk: chunk c's zsel/recip/fold
                # are emitted after chunk c+1's P matmuls so the PE never waits
                # on the DVE/scalar chain.
                def emit_zfold(c, tmp):
                    csl = slice(c * 512, (c + 1) * 512)
                    for m in range(4):
                        pz = psZ.tile([128, 512], f32, tag="pz", name=f"pz{c}_{m}")
                        nc.tensor.matmul(pz, zsel_s, tmp[:, m, :], start=True, stop=True)
                        nc.scalar.activation(pz, pz, AF.Ln)
                        zt = ztp.tile([128, 512], f32, tag="zt")
                        nc.scalar.activation(zt, pz, AF.Exp, scale=negone[:, :])
                        nc.vector.tensor_mul(qtb[:, m, csl], qtb[:, m, csl], zt)

                prev_fold = None
                for c in range(NCH):
                    csl = slice(c * 512, (c + 1) * 512)
                    Ps = [psP.tile([128, 512], f32, tag="P", name=f"P{c}_{m}")
                          for m in range(4)]
                    for m in range(4):
                        nc.tensor.matmul(Ps[m], sel_f_s[0:64, :], s_oth[:, csl],
                                         start=True, stop=False, skip_group_check=True)
                    for mp in range(3):
                        for m in range(mp + 1, 4):
                            nc.tensor.matmul(Ps[m], sel_all_s, kt[:, mp, csl],
                                             start=False, stop=False, skip_group_check=True)
                    for m in range(4):
                        nc.tensor.matmul(Ps[m], sel_last_s, kt[:, m, csl],
                                         start=False, stop=True, skip_group_check=True)
                    tmp = tmpp.tile([128, 4, 512], bf16, tag="tmp")
                    for m in range(4):
                        nc.vector.tensor_mul(tmp[:, m, :], qtb[:, m, csl], Ps[m])
                    if prev_fold is not None:
                        emit_zfold(*prev_fold)
                    prev_fold = (c, tmp)
                emit_zfold(*prev_fold)

            if "d" not in phases:
                yd = ystp.tile([128, D], bf16, tag="yst")
                nc.vector.memset(yd, 0.1)
                for tt in range(NT):
                    nc.sync.dma_start(out=y[tt * 128:(tt + 1) * 128, :], in_=yd)

            # ---- Phase D: two-level causal scan (256-token superchunks)
            # + out projection. Intra-superchunk attention covers the s-chunk
            # diagonal (triu mask) and the following 128 tokens (no mask);
            # the 64x64 per-head state S advances once per superchunk.
            if "d" in phases:
              with tc.tile_pool(name="psPA", bufs=3, space="PSUM") as psPA, \
                 tc.tile_pool(name="psPP", bufs=1, space="PSUM") as psPP, \
                 tc.tile_pool(name="psS", bufs=1, space="PSUM") as psS, \
                 tc.tile_pool(name="psY", bufs=2, space="PSUM") as psY:
                S_ps = psS.tile([128, 4, 128], f32, tag="S")
                # zero via DVE; pd matmuls then always use start=False so the
                # per-element has_written bits are never bank-wide cleared
                nc.vector.memset(S_ps, 0.0)
                s_bf_prev = None
                prev_out = None
                NSC = NT // 2

                def emit_oproj(outT_t, sc):
                    for half in range(2):
                        pys = [psY.tile([128, 512], f32, tag="py",
                                        name=f"py{sc}_{half}_{i}") for i in range(2)]
                        for kf in range(4):
                            for i in range(2):
                                nc.tensor.matmul(pys[i],
                                                 outT_t[:, kf, half * 128:(half + 1) * 128],
                                                 wo_s[:, kf, i * 512:(i + 1) * 512],
                                                 start=(kf == 0), stop=(kf == 3))
                        yst = ystp.tile([128, D], bf16, tag="yst")
                        for i in range(2):
                            nc.vector.tensor_copy(yst[:, i * 512:(i + 1) * 512], pys[i])
                        r0 = (2 * sc + half) * 128
                        nc.sync.dma_start(out=y[r0:r0 + 128, :], in_=yst)

                for sc in range(NSC):
                    blk, t8 = sc // 4, (sc % 4) * 2
                    if t8 == 0 and blk + 1 < NB:
                        emit_knat(blk + 1)
                    kn = knat_tiles[blk]
                    t0 = sc * 256
                    qsl = slice(t0, t0 + 256)
                    # attention scores. pa outputs MUST start at tile offset 0:
                    # a free-dim-offset PSUM output for this row-tiled matmul
                    # shape raises a hardware error. attn conversion split:
                    # DVE multiplies the triu diagonals, scalar copies the
                    # unmasked full block.
                    attn = attnp.tile([128, 8, 384], bf16, tag="attn")
                    for u in range(2):
                        N = 256 - u * 128
                        for pl in range(4):
                            for h in range(2):
                                j, b64 = 2 * pl + h, 64 * h
                                pat = psPA.tile([128, 256], f32, tag="pa",
                                                name=f"pa{sc}_{u}_{j}")
                                nc.tensor.matmul(
                                    pat[:, 0:N],
                                    kt[b64:b64 + 64, pl, t0 + u * 128:t0 + (u + 1) * 128],
                                    qtb[b64:b64 + 64, pl, t0 + u * 128:t0 + 256],
                                    start=True, stop=True)
                                if u == 0:
                                    nc.vector.tensor_mul(attn[:, j, 0:128],
                                                         pat[:, 0:128],
                                                         maskd_s[:, 0:128])
                                    nc.scalar.copy(attn[:, j, 128:256],
                                                   pat[:, 128:256])
                                else:
                                    nc.vector.tensor_mul(attn[:, j, 256:384],
                                                         pat[:, 0:128],
                                                         maskd_s[:, 0:128])
                    # attn @ V + q @ S
                    pp = psPP.tile([128, 4, 256], f32, tag="pp")
                    for pl in range(4):
                        for h in range(2):
                            j, b64 = 2 * pl + h, 64 * h
                            nc.tensor.matmul(pp[b64:b64 + 64, pl, 0:256],
                                             vnat[:, 2 * sc, j * 64:(j + 1) * 64],
                                             attn[:, j, 0:256],
                                             start=True, stop=False,
                                             tile_position=(0, b64))
                            nc.tensor.matmul(pp[b64:b64 + 64, pl, 128:256],
                                             vnat[:, 2 * sc + 1, j * 64:(j + 1) * 64],
                                             attn[:, j, 256:384],
                                             start=False, stop=(sc == 0),
                                             tile_position=(0, b64))
                            if sc > 0:
                                nc.tensor.matmul(pp[b64:b64 + 64, pl, 0:256],
                                                 s_bf_prev[b64:b64 + 64, pl, :],
                                                 qtb[b64:b64 + 64, pl, qsl],
                                                 start=False, stop=True,
                                                 tile_position=(b64, b64))
                    outT_t = outp.tile([128, 4, 256], bf16, tag="outT")
                    nc.scalar.copy(outT_t, pp)
                    if prev_out is not None:
                        emit_oproj(*prev_out)
                    # state update (accumulate in PSUM across all superchunks)
                    for u in range(2):
                        for pl in range(4):
                            for h in range(2):
                                j, b64 = 2 * pl + h, 64 * h
                                nc.tensor.matmul(S_ps[b64:b64 + 64, pl, 0:64],
                                                 kn[:, t8 + u, j * 64:(j + 1) * 64],
                                                 vnat[:, 2 * sc + u, j * 64:(j + 1) * 64],
                                                 start=False,
                                                 stop=(sc == NSC - 1 and u == 1),
                                                 skip_group_check=True,
                                                 tile_position=(0, b64))
                    if sc < NSC - 1:
                        s_bf = sbfp.tile([128, 4, 64], bf16, tag="sbf")
                        nc.scalar.copy(s_bf, S_ps[:, :, 0:64])
                        s_bf_prev = s_bf
                    prev_out = (outT_t, sc)
                emit_oproj(*prev_out)
    nc.finalize()
    return nc


_NC_CACHE = None


def _get_nc():
    global _NC_CACHE
    if _NC_CACHE is None:
        _NC_CACHE = build_nc()
    return _NC_CACHE


def _pack(w, kt):
    """[kt*128, N] -> [128, kt, N] contiguous."""
    return np.ascontiguousarray(w.reshape(kt, 128, -1).transpose(1, 0, 2))


def _core_inputs(x, Wq, Wk, Wv, Wo, core):
    b, s = core // 2, core % 2
    loc = np.arange(8 * s * DH, (8 * s + 8) * DH)
    c16 = lambda a: np.ascontiguousarray(a.astype(ml_dtypes.bfloat16))

    I = np.eye(64, dtype=np.float32)
    Z = np.zeros((64, 64), dtype=np.float32)
    O = np.ones((64, 64), dtype=np.float32)
    flag = float(s)
    return {
        "xt": c16(_pack(x[b].T, KD)),
        "wq": c16(_pack(Wq[:, loc], KD)),
        "wk": c16(_pack(Wk[:, loc], KD)),
        "wv": c16(_pack(Wv[:, loc], KD)),
        "wo": c16(_pack(Wo[loc, :], 4)),
        "maskt": c16(np.triu(np.ones((128, 128), dtype=np.float32))),
        "maskd": np.ascontiguousarray(np.concatenate(
            [np.triu(np.ones((128, 128), dtype=np.float32)),
             np.ones((128, 128), dtype=np.float32)], axis=1)),
        "sel_all": c16(np.block([[I, I], [I, I]])),
        "sel_last": c16(np.block([[I, I], [Z, I]])),
        "sel_f": c16(flag * np.block([[I, I], [Z, Z]])),
        "sel_sum": c16(np.block([[I, Z], [I, Z]])),
        "zsel": c16(np.block([[O, Z], [Z, O]])),
        "ident": c16(np.eye(128, dtype=np.float32)),
    }


def kernel(x, Wq, Wk, Wv, Wo, bo):
    x = np.asarray(x, dtype=np.float32)
    Wq = np.asarray(Wq, dtype=np.float32)
    Wk = np.asarray(Wk, dtype=np.float32)
    Wv = np.asarray(Wv, dtype=np.float32)
    Wo = np.asarray(Wo, dtype=np.float32)
    bo = np.asarray(bo, dtype=np.float32)

    nc = _get_nc()
    in_maps = [_core_inputs(x, Wq, Wk, Wv, Wo, core) for core in range(8)]
    try:
        res = run_bass_kernel_spmd(nc, in_maps, core_ids=list(range(8)))
    except Exception:
        # transient device/relay failures (e.g. mesh desync) recover on retry
        import time as _time
        _time.sleep(5.0)
        res = run_bass_kernel_spmd(nc, in_maps, core_ids=list(range(8)))
    out = np.zeros((B, T, D), dtype=np.float32)
    for b in range(B):
        out[b] = (res.results[2 * b]["y"].astype(np.float32)
                  + res.results[2 * b + 1]["y"].astype(np.float32))
    out += bo
    return out


# revision 21
# speedup vs baseline: 1.0119x; 1.0119x over previous
"""Trainium2 Bass kernel for nn_CausalLinearAttention (B=4, T=4096, D=1024, H=16).

Sharding: 8 cores = 4 batches x 2 head-shards (8 heads each). Per core, four
dense phases keep the PE streaming back-to-back N=512 matmuls (HAM warm):
  A: K-local projection + feature for all T, per-token local head-sum s_own,
     ONE AllReduce (pairwise) for the cross-shard head-cumsum base.
  B: Q projection + feature, V projection (token-major) for all T.
  C: normalizer fold: head-prefix sums R built in 4 PSUM banks by pure-MM
     accumulation (no PE<->DVE ping-pong), z = 1/(q . R) folded into Q.
  D: chunked causal scan; K transposed token-major via DMA-XBAR transposes
     (off the PE), per-head 64x64 state accumulated in a dedicated PSUM bank,
     output projection interleaved per 128-token chunk as dense filler.
Host sums the two head-shard partials per batch and adds the bias.

Self-contained: hardcodes shapes; only needs the concourse repo on sys.path.
"""
import sys

for _p in ("/opt/trn_rl_repo", "/root/.axon_site/_ro/trn_rl_repo"):
    if _p not in sys.path:
        sys.path.append(_p)

import numpy as np
import ml_dtypes

import concourse.bass as bass
from concourse import bacc, mybir
from concourse.bass_utils import run_bass_kernel_spmd
from concourse.tile import TileContext

bf16 = mybir.dt.bfloat16
f32 = mybir.dt.float32
AF = mybir.ActivationFunctionType
ALU = mybir.AluOpType

B, T, D = 4, 4096, 1024
H, DH = 16, 64
KD = D // 128      # 8 contraction tiles over d_model
NT = T // 128      # 32 token tiles
NB = 4             # 1024-token blocks
TB = T // NB
NCH = T // 512     # 8 512-token chunks


def build_nc(debug=False, phases="abcd", use_xbar=True, dvar="full"):
    nc = bacc.Bacc("TRN2", target_bir_lowering=False, debug=debug)

    xt = nc.dram_tensor("xt", [128, KD, T], bf16, kind="ExternalInput")
    wq = nc.dram_tensor("wq", [128, KD, 512], bf16, kind="ExternalInput")
    wk = nc.dram_tensor("wk", [128, KD, 512], bf16, kind="ExternalInput")
    wv = nc.dram_tensor("wv", [128, KD, 512], bf16, kind="ExternalInput")
    wo = nc.dram_tensor("wo", [128, 4, D], bf16, kind="ExternalInput")
    maskt_d = nc.dram_tensor("maskt", [128, 128], bf16, kind="ExternalInput")
    maskd_d = nc.dram_tensor("maskd", [128, 256], f32, kind="ExternalInput")
    sel_all_d = nc.dram_tensor("sel_all", [128, 128], bf16, kind="ExternalInput")
    sel_last_d = nc.dram_tensor("sel_last", [128, 128], bf16, kind="ExternalInput")
    sel_f_d = nc.dram_tensor("sel_f", [128, 128], bf16, kind="ExternalInput")
    sel_sum_d = nc.dram_tensor("sel_sum", [128, 128], bf16, kind="ExternalInput")
    zsel_d = nc.dram_tensor("zsel", [128, 128], bf16, kind="ExternalInput")
    ident_d = nc.dram_tensor("ident", [128, 128], bf16, kind="ExternalInput")
    y = nc.dram_tensor("y", [T, D], bf16, kind="ExternalOutput")

    with TileContext(nc) as tc:
      with tc.tile_pool(name="consts", bufs=1) as consts, \
           tc.tile_pool(name="big", bufs=1) as big, \
           tc.tile_pool(name="dram", bufs=1, space="DRAM") as dram:
        wk_s = consts.tile([128, KD, 512], bf16, tag="wk_s")
        wq_s = consts.tile([128, KD, 512], bf16, tag="wq_s")
        wv_s = consts.tile([128, KD, 512], bf16, tag="wv_s")
        wo_s = consts.tile([128, 4, D], bf16, tag="wo_s")
        mask_s = consts.tile([128, 128], bf16, tag="mask_s")
        maskd_s = consts.tile([128, 256], f32, tag="maskd_s")
        sel_all_s = consts.tile([128, 128], bf16, tag="sel_all_s")
        sel_last_s = consts.tile([128, 128], bf16, tag="sel_last_s")
        sel_f_s = consts.tile([128, 128], bf16, tag="sel_f_s")
        sel_sum_s = consts.tile([128, 128], bf16, tag="sel_sum_s")
        zsel_s = consts.tile([128, 128], bf16, tag="zsel_s")
        ident_s = consts.tile([128, 128], bf16, tag="ident_s")
        for dst, src in ((wk_s, wk), (wq_s, wq), (wv_s, wv), (wo_s, wo),
                         (mask_s, maskt_d), (maskd_s, maskd_d), (sel_all_s, sel_all_d),
                         (sel_last_s, sel_last_d), (sel_f_s, sel_f_d),
                         (sel_sum_s, sel_sum_d), (zsel_s, zsel_d),
                         (ident_s, ident_d)):
            nc.gpsimd.dma_start(out=dst, in_=src[tuple([slice(None)] * len(src.shape))])

        negone = consts.tile([128, 1], f32, tag="negone")
        nc.vector.memset(negone, -1.0)
        kt = big.tile([128, 4, T], bf16, tag="kt")
        qtb = big.tile([128, 4, T], bf16, tag="qtb")
        vnat = big.tile([128, NT, 512], bf16, tag="vnat")
        s_own = big.tile([64, T], bf16, tag="s_own")
        s_tot = big.tile([64, T], bf16, tag="s_tot")
        s_oth = big.tile([64, T], bf16, tag="s_oth")

        ktd = dram.tile([512, T], bf16, tag="ktd")
        cc_in = dram.tile([64, T], bf16, tag="cc_in")
        cc_out = dram.tile([64, T], bf16, tag="cc_out")

        # ================= Phases A & B =================
        with tc.tile_pool(name="xtp", bufs=2) as xtp, \
             tc.tile_pool(name="fe", bufs=3) as fe, \
             tc.tile_pool(name="psA", bufs=4, space="PSUM") as psA, \
             tc.tile_pool(name="psSO", bufs=2, space="PSUM") as psSO:

            def feature(psum_ap, out_ap):
                e = fe.tile([128, 512], f32, tag="e")
                nc.scalar.activation(e, psum_ap, AF.Exp)
                e2 = fe.tile([128, 512], bf16, tag="e2")
                nc.vector.tensor_scalar_min(e2, e, 1.0)
                nc.vector.scalar_tensor_tensor(out=out_ap, in0=psum_ap, scalar=0.0,
                                               in1=e2, op0=ALU.max, op1=ALU.add)

            def load_x(blk):
                xtb = xtp.tile([128, KD, TB], bf16, tag="xtb", name=f"xtb{blk}")
                for nch in range(2):
                    nc.sync.dma_start(out=xtb[:, :, nch * 512:(nch + 1) * 512],
                                      in_=xt[:, :, blk * TB + nch * 512: blk * TB + (nch + 1) * 512])
                return xtb

            def proj_block(blk, w_s, out_t, xtb):
                """feature(x @ W) for one 1024-token block, feat-major out."""
                for m in range(4):
                    pks = [psA.tile([128, 512], f32, tag="pk", name=f"pk{blk}_{m}_{i}")
                           for i in range(2)]
                    for k in range(KD):
                        for nch in range(2):
                            nc.tensor.matmul(pks[nch], w_s[:, k, m * 128:(m + 1) * 128],
                                             xtb[:, k, nch * 512:(nch + 1) * 512],
                                             start=(k == 0), stop=(k == KD - 1))
                    for nch in range(2):
                        csl = slice(blk * TB + nch * 512, blk * TB + (nch + 1) * 512)
                        feature(pks[nch], out_t[:, m, csl])
                return xtb

            # ---- Phase A: K + s_own, fire AllReduce
            xtb_cur = load_x(0)
            for blk in range(NB):
                xtb_nxt = load_x(blk + 1) if blk + 1 < NB else None
                proj_block(blk, wk_s, kt, xtb_cur)
                xtb_cur = xtb_nxt
                for nch in range(2):
                    csl = slice(blk * TB + nch * 512, blk * TB + (nch + 1) * 512)
                    so = psSO.tile([64, 512], f32, tag="so")
                    for m in range(4):
                        nc.tensor.matmul(so, sel_sum_s[:, 0:64], kt[:, m, csl],
                                         start=(m == 0), stop=(m == 3))
                    nc.scalar.copy(s_own[:, csl], so)
                bsl = slice(blk * TB, (blk + 1) * TB)
                nc.sync.dma_start(
                    out=ktd[:, bsl].rearrange("(m p) t -> p m t", p=128),
                    in_=kt[:, :, bsl])
            xtb_cur = load_x(0)
            nc.sync.dma_start(out=cc_in[:, :], in_=s_own)
            nc.gpsimd.collective_compute(
                "AllReduce", ALU.add,
                replica_groups=[[0, 1], [2, 3], [4, 5], [6, 7]],
                ins=[cc_in[:, :]], outs=[cc_out[:, :]])

            # ---- Phase B: Q + V
            for blk in range(NB):
                xtb_nxt = load_x(blk + 1) if blk + 1 < NB else None
                xtb = xtb_cur
                proj_block(blk, wq_s, qtb, xtb)
                xtb_cur = xtb_nxt
                for tt in range(8):
                    pv = psA.tile([128, 512], f32, tag="pk", name=f"pv{blk}_{tt}")
                    for k in range(KD):
                        nc.tensor.matmul(pv, xtb[:, k, tt * 128:(tt + 1) * 128],
                                         wv_s[:, k, :], start=(k == 0), stop=(k == KD - 1))
                    nc.scalar.copy(vnat[:, blk * 8 + tt, :], pv)

            # cc readback (sync queue: after all B loads so nothing stalls behind it)
            nc.sync.dma_start(out=s_tot, in_=cc_out[:, :])
            nc.vector.tensor_sub(s_oth, s_tot, s_own)

        if "c" not in phases:
            with tc.tile_pool(name="dummy", bufs=1) as dummy:
                yd = dummy.tile([128, D], bf16, tag="yd")
                nc.vector.memset(yd, 0.1)
                for tt in range(NT):
                    nc.sync.dma_start(out=y[tt * 128:(tt + 1) * 128, :], in_=yd)
            skip_cd = True
        else:
            skip_cd = False

        # ================= Phases C & D =================
        if not skip_cd:
          with tc.tile_pool(name="knatp", bufs=2) as knatp, \
             tc.tile_pool(name="tmpp", bufs=2) as tmpp, \
             tc.tile_pool(name="ztp", bufs=2) as ztp, \
             tc.tile_pool(name="attnp", bufs=2) as attnp, \
             tc.tile_pool(name="outp", bufs=2) as outp, \
             tc.tile_pool(name="sbfp", bufs=2) as sbfp, \
             tc.tile_pool(name="ystp", bufs=2) as ystp:

            knat_tiles = [None] * NB

            def emit_knat(blk):
                kn = knatp.tile([128, 8, 512], bf16, tag="knat", name=f"knat{blk}")
                for t8 in range(8):
                    tsl = slice(blk * TB + t8 * 128, blk * TB + (t8 + 1) * 128)
                    nc.sync.dma_start_transpose(out=kn[:, t8, :], in_=ktd[:, tsl])
                knat_tiles[blk] = kn

            if use_xbar:
                emit_knat(0)

            # ---- Phase C: z fold into Q
            with tc.tile_pool(name="psP", bufs=4, space="PSUM") as psP, \
                 tc.tile_pool(name="psZ", bufs=2, space="PSUM") as psZ:
                for c in range(NCH):
                    csl = slice(c * 512, (c + 1) * 512)
                    Ps = [psP.tile([128, 512], f32, tag="P", name=f"P{c}_{m}")
                          for m in range(4)]
                    for m in range(4):
                        nc.tensor.matmul(Ps[m], sel_f_s[0:64, :], s_oth[:, csl],
                                         start=True, stop=False, skip_group_check=True)
                    for mp in range(3):
                        for m in range(mp + 1, 4):
                            nc.tensor.matmul(Ps[m], sel_all_s, kt[:, mp, csl],
                                             start=False, stop=False, skip_group_check=True)
                    for m in range(4):
                        nc.tensor.matmul(Ps[m], sel_last_s, kt[:, m, csl],
                                         start=False, stop=True, skip_group_check=True)
                    tmp = tmpp.tile([128, 4, 512], bf16, tag="tmp")
                    for m in range(4):
                        nc.vector.tensor_mul(tmp[:, m, :], qtb[:, m, csl], Ps[m])
                    for m in range(4):
                        pz = psZ.tile([128, 512], f32, tag="pz", name=f"pz{c}_{m}")
                        nc.tensor.matmul(pz, zsel_s, tmp[:, m, :], start=True, stop=True)
                        nc.scalar.activation(pz, pz, AF.Ln)
                        zt = ztp.tile([128, 512], f32, tag="zt")
                        nc.scalar.activation(zt, pz, AF.Exp, scale=negone[:, :])
                        nc.vector.tensor_mul(qtb[:, m, csl], qtb[:, m, csl], zt)

            if "d" not in phases:
                yd = ystp.tile([128, D], bf16, tag="yst")
                nc.vector.memset(yd, 0.1)
                for tt in range(NT):
                    nc.sync.dma_start(out=y[tt * 128:(tt + 1) * 128, :], in_=yd)

            # ---- Phase D: two-level causal scan (256-token superchunks)
            # + out projection. Intra-superchunk attention covers the s-chunk
            # diagonal (triu mask) and the following 128 tokens (no mask);
            # the 64x64 per-head state S advances once per superchunk.
            if "d" in phases:
              with tc.tile_pool(name="psPA", bufs=3, space="PSUM") as psPA, \
                 tc.tile_pool(name="psPP", bufs=1, space="PSUM") as psPP, \
                 tc.tile_pool(name="psS", bufs=1, space="PSUM") as psS, \
                 tc.tile_pool(name="psY", bufs=2, space="PSUM") as psY:
                S_ps = psS.tile([128, 4, 128], f32, tag="S")
                # zero via DVE; pd matmuls then always use start=False so the
                # per-element has_written bits are never bank-wide cleared
                nc.vector.memset(S_ps, 0.0)
                s_bf_prev = None
                prev_out = None
                NSC = NT // 2

                def emit_oproj(outT_t, sc):
                    for half in range(2):
                        pys = [psY.tile([128, 512], f32, tag="py",
                                        name=f"py{sc}_{half}_{i}") for i in range(2)]
                        for kf in range(4):
                            for i in range(2):
                                nc.tensor.matmul(pys[i],
                                                 outT_t[:, kf, half * 128:(half + 1) * 128],
                                                 wo_s[:, kf, i * 512:(i + 1) * 512],
                                                 start=(kf == 0), stop=(kf == 3))
                        yst = ystp.tile([128, D], bf16, tag="yst")
                        for i in range(2):
                            nc.vector.tensor_copy(yst[:, i * 512:(i + 1) * 512], pys[i])
                        r0 = (2 * sc + half) * 128
                        nc.sync.dma_start(out=y[r0:r0 + 128, :], in_=yst)

                for sc in range(NSC):
                    blk, t8 = sc // 4, (sc % 4) * 2
                    if t8 == 0 and blk + 1 < NB:
                        emit_knat(blk + 1)
                    kn = knat_tiles[blk]
                    t0 = sc * 256
                    qsl = slice(t0, t0 + 256)
                    # attention scores. pa outputs MUST start at tile offset 0:
                    # a free-dim-offset PSUM output for this row-tiled matmul
                    # shape raises a hardware error. attn conversion split:
                    # DVE multiplies the triu diagonals, scalar copies the
                    # unmasked full block.
                    attn = attnp.tile([128, 8, 384], bf16, tag="attn")
                    for u in range(2):
                        N = 256 - u * 128
                        for pl in range(4):
                            for h in range(2):
                                j, b64 = 2 * pl + h, 64 * h
                                pat = psPA.tile([128, 256], f32, tag="pa",
                                                name=f"pa{sc}_{u}_{j}")
                                nc.tensor.matmul(
                                    pat[:, 0:N],
                                    kt[b64:b64 + 64, pl, t0 + u * 128:t0 + (u + 1) * 128],
                                    qtb[b64:b64 + 64, pl, t0 + u * 128:t0 + 256],
                                    start=True, stop=True)
                                if u == 0:
                                    nc.vector.tensor_mul(attn[:, j, 0:128],
                                                         pat[:, 0:128],
                                                         maskd_s[:, 0:128])
                                    nc.scalar.copy(attn[:, j, 128:256],
                                                   pat[:, 128:256])
                                else:
                                    nc.vector.tensor_mul(attn[:, j, 256:384],
                                                         pat[:, 0:128],
                                                         maskd_s[:, 0:128])
                    # attn @ V + q @ S
                    pp = psPP.tile([128, 4, 256], f32, tag="pp")
                    for pl in range(4):
                        for h in range(2):
                            j, b64 = 2 * pl + h, 64 * h
                            nc.tensor.matmul(pp[b64:b64 + 64, pl, 0:256],
                                             vnat[:, 2 * sc, j * 64:(j + 1) * 64],
                                             attn[:, j, 0:256],
                                             start=True, stop=False,
                                             tile_position=(0, b64))
                            nc.tensor.matmul(pp[b64:b64 + 64, pl, 128:256],
                                             vnat[:, 2 * sc + 1, j * 64:(j + 1) * 64],
                                             attn[:, j, 256:384],
                                             start=False, stop=(sc == 0),
                                             tile_position=(0, b64))
                            if sc > 0:
                                nc.tensor.matmul(pp[b64:b64 + 64, pl, 0:256],
                                                 s_bf_prev[b64:b64 + 64, pl, :],
                                                 qtb[b64:b64 + 64, pl, qsl],
                                                 start=False, stop=True,
                                                 tile_position=(b64, b64))
                    outT_t = outp.tile([128, 4, 256], bf16, tag="outT")
                    nc.scalar.copy(outT_t, pp)
                    if prev_out is not None:
                        emit_oproj(*prev_out)
                    # state update (accumulate in PSUM across all superchunks)
                    for u in range(2):
                        for pl in range(4):
                            for h in range(2):
                                j, b64 = 2 * pl + h, 64 * h
                                nc.tensor.matmul(S_ps[b64:b64 + 64, pl, 0:64],
                                                 kn[:, t8 + u, j * 64:(j + 1) * 64],
                                                 vnat[:, 2 * sc + u, j * 64:(j + 1) * 64],
                                                 start=False,
                                                 stop=(sc == NSC - 1 and u == 1),
                                                 skip_group_check=True,
                                                 tile_position=(0, b64))
                    if sc < NSC - 1:
                        s_bf = sbfp.tile([128, 4, 64], bf16, tag="sbf")
                        nc.scalar.copy(s_bf, S_ps[:, :, 0:64])
                        s_bf_prev = s_bf
                    prev_out = (outT_t, sc)
                emit_oproj(*prev_out)
    nc.finalize()
    return nc


_NC_CACHE = None


def _get_nc():
    global _NC_CACHE
    if _NC_CACHE is None:
        _NC_CACHE = build_nc()
    return _NC_CACHE


def _pack(w, kt):
    """[kt*128, N] -> [128, kt, N] contiguous."""
    return np.ascontiguousarray(w.reshape(kt, 128, -1).transpose(1, 0, 2))


def _core_inputs(x, Wq, Wk, Wv, Wo, core):
    b, s = core // 2, core % 2
    loc = np.arange(8 * s * DH, (8 * s + 8) * DH)
    c16 = lambda a: np.ascontiguousarray(a.astype(ml_dtypes.bfloat16))

    I = np.eye(64, dtype=np.float32)
    Z = np.zeros((64, 64), dtype=np.float32)
    O = np.ones((64, 64), dtype=np.float32)
    flag = float(s)
    return {
        "xt": c16(_pack(x[b].T, KD)),
        "wq": c16(_pack(Wq[:, loc], KD)),
        "wk": c16(_pack(Wk[:, loc], KD)),
        "wv": c16(_pack(Wv[:, loc], KD)),
        "wo": c16(_pack(Wo[loc, :], 4)),
        "maskt": c16(np.triu(np.ones((128, 128), dtype=np.float32))),
        "maskd": np.ascontiguousarray(np.concatenate(
            [np.triu(np.ones((128, 128), dtype=np.float32)),
             np.ones((128, 128), dtype=np.float32)], axis=1)),
        "sel_all": c16(np.block([[I, I], [I, I]])),
        "sel_last": c16(np.block([[I, I], [Z, I]])),
        "sel_f": c16(flag * np.block([[I, I], [Z, Z]])),
        "sel_sum": c16(np.block([[I, Z], [I, Z]])),
        "zsel": c16(np.block([[O, Z], [Z, O]])),
        "ident": c16(np.eye(128, dtype=np.float32)),
    }


def kernel(x, Wq, Wk, Wv, Wo, bo):
    x = np.asarray(x, dtype=np.float32)
    Wq = np.asarray(Wq, dtype=np.float32)
    Wk = np.asarray(Wk, dtype=np.float32)
    Wv = np.asarray(Wv, dtype=np.float32)
    Wo = np.asarray(Wo, dtype=np.float32)
    bo = np.asarray(bo, dtype=np.float32)

    nc = _get_nc()
    in_maps = [_core_inputs(x, Wq, Wk, Wv, Wo, core) for core in range(8)]
    try:
        res = run_bass_kernel_spmd(nc, in_maps, core_ids=list(range(8)))
    except Exception:
        # transient device/relay failures (e.g. mesh desync) recover on retry
        import time as _time
        _time.sleep(5.0)
        res = run_bass_kernel_spmd(nc, in_maps, core_ids=list(range(8)))
    out = np.zeros((B, T, D), dtype=np.float32)
    for b in range(B):
        out[b] = (res.results[2 * b]["y"].astype(np.float32)
                  + res.results[2 * b + 1]["y"].astype(np.float32))
    out += bo
    return out


# revision 22
# speedup vs baseline: 1.1097x; 1.0966x over previous
"""Trainium2 Bass kernel for nn_CausalLinearAttention (B=4, T=4096, D=1024, H=16).

Sharding: 8 cores = 4 batches x 2 head-shards (8 heads each). Per core, four
dense phases keep the PE streaming back-to-back N=512 matmuls (HAM warm):
  A: K-local projection + feature for all T, per-token local head-sum s_own,
     ONE AllReduce (pairwise) for the cross-shard head-cumsum base.
  B: Q projection + feature, V projection (token-major) for all T.
  C: normalizer fold: head-prefix sums R built in 4 PSUM banks by pure-MM
     accumulation (no PE<->DVE ping-pong), z = 1/(q . R) folded into Q.
  D: chunked causal scan; K transposed token-major via DMA-XBAR transposes
     (off the PE), per-head 64x64 state accumulated in a dedicated PSUM bank,
     output projection interleaved per 128-token chunk as dense filler.
Host sums the two head-shard partials per batch and adds the bias.

Self-contained: hardcodes shapes; only needs the concourse repo on sys.path.
"""
import sys

for _p in ("/opt/trn_rl_repo", "/root/.axon_site/_ro/trn_rl_repo"):
    if _p not in sys.path:
        sys.path.append(_p)

import numpy as np
import ml_dtypes

import concourse.bass as bass
from concourse import bacc, mybir
from concourse.bass_utils import run_bass_kernel_spmd
from concourse.tile import TileContext

bf16 = mybir.dt.bfloat16
f32 = mybir.dt.float32
AF = mybir.ActivationFunctionType
ALU = mybir.AluOpType

B, T, D = 4, 4096, 1024
H, DH = 16, 64
KD = D // 128      # 8 contraction tiles over d_model
NT = T // 128      # 32 token tiles
NB = 4             # 1024-token blocks
TB = T // NB
NCH = T // 512     # 8 512-token chunks


def build_nc(debug=False, phases="abcd", use_xbar=True, dvar="full"):
    nc = bacc.Bacc("TRN2", target_bir_lowering=False, debug=debug)

    xt = nc.dram_tensor("xt", [128, KD, T], bf16, kind="ExternalInput")
    wq = nc.dram_tensor("wq", [128, KD, 512], bf16, kind="ExternalInput")
    wk = nc.dram_tensor("wk", [128, KD, 512], bf16, kind="ExternalInput")
    wv = nc.dram_tensor("wv", [128, KD, 512], bf16, kind="ExternalInput")
    wo = nc.dram_tensor("wo", [128, 4, D], bf16, kind="ExternalInput")
    maskt_d = nc.dram_tensor("maskt", [128, 128], bf16, kind="ExternalInput")
    maskd_d = nc.dram_tensor("maskd", [128, 256], f32, kind="ExternalInput")
    sel_all_d = nc.dram_tensor("sel_all", [128, 128], bf16, kind="ExternalInput")
    sel_last_d = nc.dram_tensor("sel_last", [128, 128], bf16, kind="ExternalInput")
    sel_f_d = nc.dram_tensor("sel_f", [128, 128], bf16, kind="ExternalInput")
    sel_sum_d = nc.dram_tensor("sel_sum", [128, 128], bf16, kind="ExternalInput")
    zsel_d = nc.dram_tensor("zsel", [128, 128], bf16, kind="ExternalInput")
    ident_d = nc.dram_tensor("ident", [128, 128], bf16, kind="ExternalInput")
    y = nc.dram_tensor("y", [T, D], bf16, kind="ExternalOutput")

    with TileContext(nc) as tc:
      with tc.tile_pool(name="consts", bufs=1) as consts, \
           tc.tile_pool(name="big", bufs=1) as big, \
           tc.tile_pool(name="dram", bufs=1, space="DRAM") as dram:
        wk_s = consts.tile([128, KD, 512], bf16, tag="wk_s")
        wq_s = consts.tile([128, KD, 512], bf16, tag="wq_s")
        wv_s = consts.tile([128, KD, 512], bf16, tag="wv_s")
        wo_s = consts.tile([128, 4, D], bf16, tag="wo_s")
        mask_s = consts.tile([128, 128], bf16, tag="mask_s")
        maskd_s = consts.tile([128, 256], f32, tag="maskd_s")
        sel_all_s = consts.tile([128, 128], bf16, tag="sel_all_s")
        sel_last_s = consts.tile([128, 128], bf16, tag="sel_last_s")
        sel_f_s = consts.tile([128, 128], bf16, tag="sel_f_s")
        sel_sum_s = consts.tile([128, 128], bf16, tag="sel_sum_s")
        zsel_s = consts.tile([128, 128], bf16, tag="zsel_s")
        ident_s = consts.tile([128, 128], bf16, tag="ident_s")
        for dst, src in ((wk_s, wk), (wq_s, wq), (wv_s, wv), (wo_s, wo),
                         (mask_s, maskt_d), (maskd_s, maskd_d), (sel_all_s, sel_all_d),
                         (sel_last_s, sel_last_d), (sel_f_s, sel_f_d),
                         (sel_sum_s, sel_sum_d), (zsel_s, zsel_d),
                         (ident_s, ident_d)):
            nc.gpsimd.dma_start(out=dst, in_=src[tuple([slice(None)] * len(src.shape))])

        negone = consts.tile([128, 1], f32, tag="negone")
        nc.vector.memset(negone, -1.0)
        kt = big.tile([128, 4, T], bf16, tag="kt")
        qtb = big.tile([128, 4, T], bf16, tag="qtb")
        vnat = big.tile([128, NT, 512], bf16, tag="vnat")
        s_own = big.tile([64, T], bf16, tag="s_own")
        s_tot = big.tile([64, T], bf16, tag="s_tot")
        s_oth = big.tile([64, T], bf16, tag="s_oth")

        ktd = dram.tile([512, T], bf16, tag="ktd")
        cc_in = dram.tile([64, T], bf16, tag="cc_in")
        cc_out = dram.tile([64, T], bf16, tag="cc_out")

        # ================= Phases A & B =================
        with tc.tile_pool(name="xtp", bufs=2) as xtp, \
             tc.tile_pool(name="fe", bufs=3) as fe, \
             tc.tile_pool(name="psA", bufs=4, space="PSUM") as psA, \
             tc.tile_pool(name="psSO", bufs=2, space="PSUM") as psSO:

            def feature(psum_ap, out_ap):
                e = fe.tile([128, 512], f32, tag="e")
                nc.scalar.activation(e, psum_ap, AF.Exp)
                e2 = fe.tile([128, 512], bf16, tag="e2")
                nc.vector.tensor_scalar_min(e2, e, 1.0)
                nc.vector.scalar_tensor_tensor(out=out_ap, in0=psum_ap, scalar=0.0,
                                               in1=e2, op0=ALU.max, op1=ALU.add)

            def load_x(blk):
                xtb = xtp.tile([128, KD, TB], bf16, tag="xtb", name=f"xtb{blk}")
                for nch in range(2):
                    nc.sync.dma_start(out=xtb[:, :, nch * 512:(nch + 1) * 512],
                                      in_=xt[:, :, blk * TB + nch * 512: blk * TB + (nch + 1) * 512])
                return xtb

            def proj_block(blk, w_s, out_t, xtb):
                """feature(x @ W) for one 1024-token block, feat-major out."""
                for m in range(4):
                    pks = [psA.tile([128, 512], f32, tag="pk", name=f"pk{blk}_{m}_{i}")
                           for i in range(2)]
                    for k in range(KD):
                        for nch in range(2):
                            nc.tensor.matmul(pks[nch], w_s[:, k, m * 128:(m + 1) * 128],
                                             xtb[:, k, nch * 512:(nch + 1) * 512],
                                             start=(k == 0), stop=(k == KD - 1))
                    for nch in range(2):
                        csl = slice(blk * TB + nch * 512, blk * TB + (nch + 1) * 512)
                        feature(pks[nch], out_t[:, m, csl])
                return xtb

            # ---- Phase A: K + s_own, fire AllReduce
            xtb_cur = load_x(0)
            for blk in range(NB):
                xtb_nxt = load_x(blk + 1) if blk + 1 < NB else None
                proj_block(blk, wk_s, kt, xtb_cur)
                xtb_cur = xtb_nxt
                for nch in range(2):
                    csl = slice(blk * TB + nch * 512, blk * TB + (nch + 1) * 512)
                    so = psSO.tile([64, 512], f32, tag="so")
                    for m in range(4):
                        nc.tensor.matmul(so, sel_sum_s[:, 0:64], kt[:, m, csl],
                                         start=(m == 0), stop=(m == 3))
                    nc.scalar.copy(s_own[:, csl], so)
                bsl = slice(blk * TB, (blk + 1) * TB)
                nc.sync.dma_start(
                    out=ktd[:, bsl].rearrange("(m p) t -> p m t", p=128),
                    in_=kt[:, :, bsl])
            xtb_cur = load_x(0)
            nc.sync.dma_start(out=cc_in[:, :], in_=s_own)
            nc.gpsimd.collective_compute(
                "AllReduce", ALU.add,
                replica_groups=[[0, 1], [2, 3], [4, 5], [6, 7]],
                ins=[cc_in[:, :]], outs=[cc_out[:, :]])

            # ---- Phase B: Q + V
            for blk in range(NB):
                xtb_nxt = load_x(blk + 1) if blk + 1 < NB else None
                xtb = xtb_cur
                proj_block(blk, wq_s, qtb, xtb)
                xtb_cur = xtb_nxt
                for tt in range(8):
                    pv = psA.tile([128, 512], f32, tag="pk", name=f"pv{blk}_{tt}")
                    for k in range(KD):
                        nc.tensor.matmul(pv, xtb[:, k, tt * 128:(tt + 1) * 128],
                                         wv_s[:, k, :], start=(k == 0), stop=(k == KD - 1))
                    nc.scalar.copy(vnat[:, blk * 8 + tt, :], pv)

            # cc readback (sync queue: after all B loads so nothing stalls behind it)
            nc.sync.dma_start(out=s_tot, in_=cc_out[:, :])
            nc.vector.tensor_sub(s_oth, s_tot, s_own)

        if "c" not in phases:
            with tc.tile_pool(name="dummy", bufs=1) as dummy:
                yd = dummy.tile([128, D], bf16, tag="yd")
                nc.vector.memset(yd, 0.1)
                for tt in range(NT):
                    nc.sync.dma_start(out=y[tt * 128:(tt + 1) * 128, :], in_=yd)
            skip_cd = True
        else:
            skip_cd = False

        # ================= Phases C & D =================
        if not skip_cd:
          with tc.tile_pool(name="knatp", bufs=2) as knatp, \
             tc.tile_pool(name="tmpp", bufs=2) as tmpp, \
             tc.tile_pool(name="ztp", bufs=2) as ztp, \
             tc.tile_pool(name="attnp", bufs=2) as attnp, \
             tc.tile_pool(name="outp", bufs=2) as outp, \
             tc.tile_pool(name="sbfp", bufs=2) as sbfp, \
             tc.tile_pool(name="ystp", bufs=2) as ystp:

            knat_tiles = [None] * NB

            def emit_knat(blk):
                kn = knatp.tile([128, 8, 512], bf16, tag="knat", name=f"knat{blk}")
                for t8 in range(8):
                    tsl = slice(blk * TB + t8 * 128, blk * TB + (t8 + 1) * 128)
                    nc.sync.dma_start_transpose(out=kn[:, t8, :], in_=ktd[:, tsl])
                knat_tiles[blk] = kn

            if use_xbar:
                emit_knat(0)

            # ---- Phase C: z fold into Q
            with tc.tile_pool(name="psP", bufs=4, space="PSUM") as psP, \
                 tc.tile_pool(name="psZ", bufs=2, space="PSUM") as psZ:
                for c in range(NCH):
                    csl = slice(c * 512, (c + 1) * 512)
                    Ps = [psP.tile([128, 512], f32, tag="P", name=f"P{c}_{m}")
                          for m in range(4)]
                    for m in range(4):
                        nc.tensor.matmul(Ps[m], sel_f_s[0:64, :], s_oth[:, csl],
                                         start=True, stop=False, skip_group_check=True)
                    for mp in range(3):
                        for m in range(mp + 1, 4):
                            nc.tensor.matmul(Ps[m], sel_all_s, kt[:, mp, csl],
                                             start=False, stop=False, skip_group_check=True)
                    for m in range(4):
                        nc.tensor.matmul(Ps[m], sel_last_s, kt[:, m, csl],
                                         start=False, stop=True, skip_group_check=True)
                    tmp = tmpp.tile([128, 4, 512], bf16, tag="tmp")
                    for m in range(4):
                        nc.vector.tensor_mul(tmp[:, m, :], qtb[:, m, csl], Ps[m])
                    for m in range(4):
                        pz = psZ.tile([128, 512], f32, tag="pz", name=f"pz{c}_{m}")
                        nc.tensor.matmul(pz, zsel_s, tmp[:, m, :], start=True, stop=True)
                        nc.scalar.activation(pz, pz, AF.Ln)
                        zt = ztp.tile([128, 512], f32, tag="zt")
                        nc.scalar.activation(zt, pz, AF.Exp, scale=negone[:, :])
                        nc.vector.tensor_mul(qtb[:, m, csl], qtb[:, m, csl], zt)

            if "d" not in phases:
                yd = ystp.tile([128, D], bf16, tag="yst")
                nc.vector.memset(yd, 0.1)
                for tt in range(NT):
                    nc.sync.dma_start(out=y[tt * 128:(tt + 1) * 128, :], in_=yd)

            # ---- Phase D: two-level causal scan (256-token superchunks)
            # + out projection. Intra-superchunk attention covers the s-chunk
            # diagonal (triu mask) and the following 128 tokens (no mask);
            # the 64x64 per-head state S advances once per superchunk.
            if "d" in phases:
              with tc.tile_pool(name="psPA", bufs=3, space="PSUM") as psPA, \
                 tc.tile_pool(name="psPP", bufs=1, space="PSUM") as psPP, \
                 tc.tile_pool(name="psS", bufs=1, space="PSUM") as psS, \
                 tc.tile_pool(name="psY", bufs=2, space="PSUM") as psY:
                S_ps = psS.tile([128, 4, 128], f32, tag="S")
                # zero via DVE; pd matmuls then always use start=False so the
                # per-element has_written bits are never bank-wide cleared
                nc.vector.memset(S_ps, 0.0)
                s_bf_prev = None
                prev_out = None
                NSC = NT // 2

                def emit_oproj(outT_t, sc):
                    for half in range(2):
                        pys = [psY.tile([128, 512], f32, tag="py",
                                        name=f"py{sc}_{half}_{i}") for i in range(2)]
                        for kf in range(4):
                            for i in range(2):
                                nc.tensor.matmul(pys[i],
                                                 outT_t[:, kf, half * 128:(half + 1) * 128],
                                                 wo_s[:, kf, i * 512:(i + 1) * 512],
                                                 start=(kf == 0), stop=(kf == 3))
                        yst = ystp.tile([128, D], bf16, tag="yst")
                        for i in range(2):
                            nc.vector.tensor_copy(yst[:, i * 512:(i + 1) * 512], pys[i])
                        r0 = (2 * sc + half) * 128
                        nc.sync.dma_start(out=y[r0:r0 + 128, :], in_=yst)

                for sc in range(NSC):
                    blk, t8 = sc // 4, (sc % 4) * 2
                    if t8 == 0 and blk + 1 < NB:
                        emit_knat(blk + 1)
                    kn = knat_tiles[blk]
                    t0 = sc * 256
                    qsl = slice(t0, t0 + 256)
                    # attention scores. pa outputs MUST start at tile offset 0:
                    # a free-dim-offset PSUM output for this row-tiled matmul
                    # shape raises a hardware error. attn conversion split:
                    # DVE multiplies the triu diagonals, scalar copies the
                    # unmasked full block.
                    attn = attnp.tile([128, 8, 384], bf16, tag="attn")
                    for u in range(2):
                        N = 256 - u * 128
                        for pl in range(4):
                            for h in range(2):
                                j, b64 = 2 * pl + h, 64 * h
                                pat = psPA.tile([128, 256], f32, tag="pa",
                                                name=f"pa{sc}_{u}_{j}")
                                nc.tensor.matmul(
                                    pat[:, 0:N],
                                    kt[b64:b64 + 64, pl, t0 + u * 128:t0 + (u + 1) * 128],
                                    qtb[b64:b64 + 64, pl, t0 + u * 128:t0 + 256],
                                    start=True, stop=True)
                                if u == 0:
                                    nc.vector.tensor_mul(attn[:, j, 0:128],
                                                         pat[:, 0:128],
                                                         maskd_s[:, 0:128])
                                    nc.scalar.copy(attn[:, j, 128:256],
                                                   pat[:, 128:256])
                                else:
                                    nc.vector.tensor_mul(attn[:, j, 256:384],
                                                         pat[:, 0:128],
                                                         maskd_s[:, 0:128])
                    # attn @ V + q @ S. The pp bank is DVE-zeroed and all
                    # matmuls use start=False (never clearing has_written bank
                    # wide), so the h0/h1 col-group pairs can be emitted
                    # adjacently and run concurrently on the PE array.
                    pp = psPP.tile([128, 4, 256], f32, tag="pp")
                    nc.vector.memset(pp, 0.0)
                    for pl in range(4):
                        for h in range(2):
                            j, b64 = 2 * pl + h, 64 * h
                            nc.tensor.matmul(pp[b64:b64 + 64, pl, 0:256],
                                             vnat[:, 2 * sc, j * 64:(j + 1) * 64],
                                             attn[:, j, 0:256],
                                             start=False, stop=False,
                                             skip_group_check=True,
                                             tile_position=(0, b64))
                        for h in range(2):
                            j, b64 = 2 * pl + h, 64 * h
                            nc.tensor.matmul(pp[b64:b64 + 64, pl, 128:256],
                                             vnat[:, 2 * sc + 1, j * 64:(j + 1) * 64],
                                             attn[:, j, 256:384],
                                             start=False, stop=(sc == 0 and h == 1),
                                             skip_group_check=True,
                                             tile_position=(0, b64))
                        if sc > 0:
                            for h in range(2):
                                b64 = 64 * h
                                nc.tensor.matmul(pp[b64:b64 + 64, pl, 0:256],
                                                 s_bf_prev[b64:b64 + 64, pl, :],
                                                 qtb[b64:b64 + 64, pl, qsl],
                                                 start=False, stop=(h == 1),
                                                 skip_group_check=True,
                                                 tile_position=(b64, b64))
                    outT_t = outp.tile([128, 4, 256], bf16, tag="outT")
                    nc.scalar.copy(outT_t, pp)
                    if prev_out is not None:
                        emit_oproj(*prev_out)
                    # state update (accumulate in PSUM across all superchunks)
                    for u in range(2):
                        for pl in range(4):
                            for h in range(2):
                                j, b64 = 2 * pl + h, 64 * h
                                nc.tensor.matmul(S_ps[b64:b64 + 64, pl, 0:64],
                                                 kn[:, t8 + u, j * 64:(j + 1) * 64],
                                                 vnat[:, 2 * sc + u, j * 64:(j + 1) * 64],
                                                 start=False,
                                                 stop=(sc == NSC - 1 and u == 1),
                                                 skip_group_check=True,
                                                 tile_position=(0, b64))
                    if sc < NSC - 1:
                        s_bf = sbfp.tile([128, 4, 64], bf16, tag="sbf")
                        nc.scalar.copy(s_bf, S_ps[:, :, 0:64])
                        s_bf_prev = s_bf
                    prev_out = (outT_t, sc)
                emit_oproj(*prev_out)
    nc.finalize()
    return nc


_NC_CACHE = None


def _get_nc():
    global _NC_CACHE
    if _NC_CACHE is None:
        _NC_CACHE = build_nc()
    return _NC_CACHE


def _pack(w, kt):
    """[kt*128, N] -> [128, kt, N] contiguous."""
    return np.ascontiguousarray(w.reshape(kt, 128, -1).transpose(1, 0, 2))


def _core_inputs(x, Wq, Wk, Wv, Wo, core):
    b, s = core // 2, core % 2
    loc = np.arange(8 * s * DH, (8 * s + 8) * DH)
    c16 = lambda a: np.ascontiguousarray(a.astype(ml_dtypes.bfloat16))

    I = np.eye(64, dtype=np.float32)
    Z = np.zeros((64, 64), dtype=np.float32)
    O = np.ones((64, 64), dtype=np.float32)
    flag = float(s)
    return {
        "xt": c16(_pack(x[b].T, KD)),
        "wq": c16(_pack(Wq[:, loc], KD)),
        "wk": c16(_pack(Wk[:, loc], KD)),
        "wv": c16(_pack(Wv[:, loc], KD)),
        "wo": c16(_pack(Wo[loc, :], 4)),
        "maskt": c16(np.triu(np.ones((128, 128), dtype=np.float32))),
        "maskd": np.ascontiguousarray(np.concatenate(
            [np.triu(np.ones((128, 128), dtype=np.float32)),
             np.ones((128, 128), dtype=np.float32)], axis=1)),
        "sel_all": c16(np.block([[I, I], [I, I]])),
        "sel_last": c16(np.block([[I, I], [Z, I]])),
        "sel_f": c16(flag * np.block([[I, I], [Z, Z]])),
        "sel_sum": c16(np.block([[I, Z], [I, Z]])),
        "zsel": c16(np.block([[O, Z], [Z, O]])),
        "ident": c16(np.eye(128, dtype=np.float32)),
    }


def kernel(x, Wq, Wk, Wv, Wo, bo):
    x = np.asarray(x, dtype=np.float32)
    Wq = np.asarray(Wq, dtype=np.float32)
    Wk = np.asarray(Wk, dtype=np.float32)
    Wv = np.asarray(Wv, dtype=np.float32)
    Wo = np.asarray(Wo, dtype=np.float32)
    bo = np.asarray(bo, dtype=np.float32)

    nc = _get_nc()
    in_maps = [_core_inputs(x, Wq, Wk, Wv, Wo, core) for core in range(8)]
    try:
        res = run_bass_kernel_spmd(nc, in_maps, core_ids=list(range(8)))
    except Exception:
        # transient device/relay failures (e.g. mesh desync) recover on retry
        import time as _time
        _time.sleep(5.0)
        res = run_bass_kernel_spmd(nc, in_maps, core_ids=list(range(8)))
    out = np.zeros((B, T, D), dtype=np.float32)
    for b in range(B):
        out[b] = (res.results[2 * b]["y"].astype(np.float32)
                  + res.results[2 * b + 1]["y"].astype(np.float32))
    out += bo
    return out
